# revision 24
# baseline (speedup 1.0000x reference)
"""CrossModalAttention kernel for 8 Trainium2 NeuronCores.

Data-parallel over batch: B=16 -> 2 batches per core.

Math (per batch, with A=audio [N,d], V=video [M,d]):
  scores*sqrt(d) = (A Wa^T + 1 b_a^T)(V Wv^T + 1 b_v^T)^T
                 = A M V^T + (row-constant terms) + 1_n w^T
  with M = Wa^T Wv, w = V (Wv^T b_a).  Row-constant terms drop inside
  softmax, and max-subtraction is skipped: scores are ~N(0,1), far from
  fp32 exp overflow.
  attn     = exp_s / rowsum, with exp_s kept transposed [m, n]
  att_T[d,n] = sum_m video[m,d] exp_s[m,n]
  out[n,f] = att_T^T @ Wo^T * (1/rowsum[n]) + b_o

All matmuls run as fp32r (TF32-like, 1 cyc/row) with fp32 PSUM
accumulation; fp32r operands are produced by explicit rounding copies
(BIR verifier requirement).  K is always on partitions:

  video_r[m,d] <- GPSIMD round-copy of DMA'd video chunk
  videoT[d,m]  <- PE-transpose of video_r (fp32r, 1.5 cyc/row)
  aT[d1,n]  <- PE-transpose of audio slice (fp32, 2 cyc/row, cast on evict)
  gT[d2,n]  =  M[d1,d2](st)       x aT(mv)
  sT[m,n]   =  videoT[d2,m](st)   x gT(mv);   exp on ACT -> fp32r
  rowsum    :  DVE-accumulated exp chunks, one [1,n] PE reduce at end
  attT[d,n] =  video_r[m,d](st)   x exp[m](mv), 4 PSUM banks held per nt
  out[n,f]  =  attT[e,n](st)      x WoT[e,f](mv); DVE fuses *recip + b_o

Scheduling: flat software-pipelined loop over the 8 (batch, n-tile)
pairs.  Video prep (chunk DMA + round + PE transpose) is fused into
each batch's first n-tile mc-loop so the PE never sits idle long
enough for the HAM clock gate to re-throttle.  Audio/gT prep for tile
t+1 is emitted between tile t's attention and its output projection.
DMA issue queues: video on Sync, audio on Scalar, output on GpSimd.
"""

import os
from contextlib import ExitStack

import numpy as np

import concourse.bass as bass
import concourse.mybir as mybir
import concourse.tile as tile
from concourse import bacc
from concourse.bass_utils import run_bass_kernel_spmd
from concourse.masks import make_identity

B, SEQ, D = 16, 2048, 512
NCORES = 8
BL = B // NCORES          # batches per core
P = 128
DC = D // P               # 4 chunks of the model dim
MC = SEQ // P             # 16 m-chunks per batch
NTW = 512                 # n-tile width
NT = SEQ // NTW           # 4 n-tiles per batch
NSC = NTW // P            # 4 n-subchunks per n-tile
TOT = BL * NT             # flat (batch, nt) tiles per core
SCALE = 1.0 / float(np.sqrt(D))

F32 = mybir.dt.float32
FMM = mybir.dt.float32r   # matmul operand dtype


def _body(tc, ctx, has_ba=False):
    nc = tc.nc
    audio = nc.t_audio.ap()
    video = nc.t_video.ap()
    out = nc.t_out.ap()

    const = ctx.enter_context(tc.tile_pool(name="const", bufs=1))
    # PSUM: 4 att banks held per nt + 2 rotating big banks + 2 transpose
    ps_att = ctx.enter_context(tc.tile_pool(name="ps_att", bufs=4, space="PSUM"))
    ps_sc = ctx.enter_context(tc.tile_pool(name="ps_sc", bufs=2, space="PSUM"))
    ps_tp = ctx.enter_context(tc.tile_pool(name="ps_tp", bufs=2, space="PSUM"))

    # ---- constants ----
    ident = const.tile([P, P], F32, tag="ident")
    make_identity(nc, ident[:])
    ident_r = const.tile([P, P], FMM, tag="ident_r")
    nc.vector.tensor_copy(ident_r[:], ident[:])
    ones_f32 = const.tile([P, P], F32, tag="ones_f32")
    nc.gpsimd.memset(ones_f32[:], 1.0)
    ones_col = const.tile([P, 1], FMM, tag="ones_col")
    nc.vector.tensor_copy(ones_col[:], ones_f32[:, 0:1])
    ones_row = const.tile([1, P], FMM, tag="ones_row")
    nc.vector.tensor_copy(ones_row[:], ones_f32[0:1, :])

    m_sb = const.tile([P, DC, D], FMM, tag="m_sb")
    woT = const.tile([P, DC, D], FMM, tag="woT")
    wo_sb = const.tile([P, DC, D], F32, tag="wo_sb")
    bo_sb = const.tile([1, D], F32, tag="bo_sb")
    bo_bc = const.tile([P, D], F32, tag="bo_bc")
    q_bc = const.tile([P, D], F32, tag="q_bc") if has_ba else None

    # early pools so data DMAs can be issued during setup
    vtmp_pool = ctx.enter_context(tc.tile_pool(name="vtmp", bufs=6))
    nt_pool = ctx.enter_context(tc.tile_pool(name="nt", bufs=2))

    vtmp = [[None] * MC for _ in range(BL)]
    a_sb = [None] * TOT
    a_r = [None] * TOT
    aT = [None] * TOT
    gT = [None] * TOT

    def vid_dma(b):
        """Issue video chunk DMAs for batch b.  Batch 0 alternates the
        sync/scalar rings (scalar is free until the first outputs) so
        chunks land twice as fast during the cold start; batch 1 stays
        on sync to keep the scalar queue clear for output DMAs."""
        b0 = b * SEQ
        for mc in range(MC):
            vt = vtmp_pool.tile([P, D], F32, tag="vtmp", name="vtmp")
            if b == 0 and mc % 2 == 1:
                nc.scalar.dma_start(vt[:],
                                    video[b0 + mc * P:b0 + (mc + 1) * P, :])
            else:
                nc.sync.dma_start(vt[:],
                                  video[b0 + mc * P:b0 + (mc + 1) * P, :])
            vtmp[b][mc] = vt

    def aud_dma(t):
        """Issue audio DMAs for flat tile t (gpsimd queue)."""
        a_sb[t] = nt_pool.tile([P, NSC, D], F32, tag="a_sb", bufs=1,
                               name="a_sb")
        a_r[t] = nt_pool.tile([P, NSC, D], FMM, tag="a_r", bufs=1, name="a_r")
        n0 = (t // NT) * SEQ + (t % NT) * NTW
        for rc in range(NSC):
            nc.gpsimd.dma_start(a_sb[t][:, rc, :],
                                audio[n0 + rc * P:n0 + (rc + 1) * P, :])

    def aud_round(t):
        """DVE round-casts audio to fp32r (emitted once the DMA is old)."""
        for rc in range(NSC):
            nc.vector.tensor_copy(a_r[t][:, rc, :], a_sb[t][:, rc, :])
        a_sb[t] = None

    # ---- setup: weights, M = Wa^T Wv, WoT, biases ----
    # wa on sync / wv on scalar so both rings run in parallel; the data
    # DMAs (video chunks on sync, audio on scalar) queue right behind.
    with tc.tile_pool(name="setup", bufs=1) as setup:
        wa_sb = setup.tile([P, DC, D], F32, tag="wa_sb")
        wv_sb = setup.tile([P, DC, D], F32, tag="wv_sb")
        # per-chunk weight DMAs so the round copies / M matmuls can start
        # on chunk 0 while later chunks are still in flight
        for ec in range(DC):
            nc.sync.dma_start(wa_sb[:, ec, :],
                              nc.t_wa.ap()[ec * P:(ec + 1) * P, :])
            nc.scalar.dma_start(wv_sb[:, ec, :],
                                nc.t_wv.ap()[ec * P:(ec + 1) * P, :])
        vid_dma(0)
        aud_dma(0)
        nc.gpsimd.dma_start(bo_sb[:], nc.t_bo.ap().rearrange("(o d) -> o d", o=1))
        for fc in range(DC):
            nc.gpsimd.dma_start(wo_sb[:, fc, :],
                                nc.t_wo.ap()[fc * P:(fc + 1) * P, :])

        # per-ec round copies pipelined with the M accumulation (ps_att)
        wa_r = setup.tile([P, DC, D], FMM, tag="wa_r")
        wv_r = setup.tile([P, DC, D], FMM, tag="wv_r")
        pm = [ps_att.tile([P, NTW], F32, tag="att", name="pm")
              for _ in range(DC)]
        for ec in range(DC):
            nc.scalar.copy(wa_r[:, ec, :], wa_sb[:, ec, :])
            nc.vector.tensor_copy(wv_r[:, ec, :], wv_sb[:, ec, :])
            for d1c in range(DC):
                nc.tensor.matmul(pm[d1c][:], wa_r[:, ec, d1c * P:(d1c + 1) * P],
                                 wv_r[:, ec, :], start=(ec == 0),
                                 stop=(ec == DC - 1))
        for d1c in range(DC):
            nc.scalar.copy(m_sb[:, d1c, :], pm[d1c][:])

        if has_ba:
            ba_sb = setup.tile([P, DC], F32, tag="ba_sb")
            nc.sync.dma_start(ba_sb[:], nc.t_ba.ap().rearrange("(c p) -> p c", p=P))
            ba_r = setup.tile([P, DC], FMM, tag="ba_r")
            nc.vector.tensor_copy(ba_r[:], ba_sb[:])
            # q = Wv^T b_a [1, d] -> broadcast to q_bc [128, d], pre-scaled
            pqf = ps_sc.tile([P, NTW], F32, tag="sc")
            for ec in range(DC):
                nc.tensor.matmul(pqf[0:1, :], ba_r[:, ec:ec + 1],
                                 wv_r[:, ec, :],
                                 start=(ec == 0), stop=(ec == DC - 1))
            q_row = setup.tile([1, D], FMM, tag="q_row")
            nc.scalar.mul(q_row[:], pqf[0:1, :], SCALE)
            pqb = ps_sc.tile([P, NTW], F32, tag="sc")
            nc.tensor.matmul(pqb[:], ones_row[:], q_row[:], start=True, stop=True)
            nc.vector.tensor_copy(q_bc[:], pqb[:])

    # ---- main pools ----
    vid = ctx.enter_context(tc.tile_pool(name="vid", bufs=1))
    exp_pool = ctx.enter_context(tc.tile_pool(name="expp", bufs=4))
    outp = ctx.enter_context(tc.tile_pool(name="outp", bufs=2))
    small = ctx.enter_context(tc.tile_pool(name="small", bufs=2))

    # per-batch video buffers (round-copied fp32r, double-buffered)
    video_r = [None] * BL
    videoT_t = vid.tile([P, DC, SEQ], FMM, tag="videoT")
    w_col = [None] * BL

    bo_row_t = small.tile([1, D], FMM, tag="bo_row", bufs=1, name="bo_row")

    def wo_build():
        """woT transposes + b_o broadcast; emitted after body(0) so the
        PE is already warm and the wo DMA is long done."""
        for fc in range(DC):
            ptb = ps_tp.tile([P, NTW], F32, tag="tp")
            for ec in range(DC):
                nc.tensor.transpose(ptb[:, ec * P:(ec + 1) * P],
                                    wo_sb[:, fc, ec * P:(ec + 1) * P],
                                    ident[:])
            nc.vector.tensor_copy(
                woT[:, :, fc * P:(fc + 1) * P],
                ptb[:].rearrange("p (c x) -> p c x", c=DC))
        nc.vector.tensor_copy(bo_row_t[:], bo_sb[:])
        pbb = ps_sc.tile([P, NTW], F32, tag="sc")
        nc.tensor.matmul(pbb[:], ones_row[:], bo_row_t[:], start=True, stop=True)
        nc.vector.tensor_copy(bo_bc[:], pbb[:])

    def vid_alloc(b):
        video_r[b] = vid.tile([P, MC, D], FMM, tag="video_r", bufs=2,
                              name="video_r")
        if has_ba:
            w_col[b] = vid.tile([P, MC, 1], F32, tag="w_col", bufs=2,
                                name="w_col")

    def prep(t):
        """Audio transposes + gT for flat tile t (PE work).  The 4 dc
        transposes of one row-chunk go into a single full PSUM bank and
        leave with one strided eviction: back-to-back PE transposes and
        4x fewer DVE round-trips on the ps_tp ring."""
        aT[t] = nt_pool.tile([P, DC, NTW], FMM, tag="aT", bufs=1, name="aT")
        for rc in range(NSC):
            ptb = ps_tp.tile([P, NTW], F32, tag="tp")
            for dc in range(DC):
                nc.tensor.transpose(ptb[:, dc * P:(dc + 1) * P].bitcast(FMM),
                                    a_r[t][:, rc, dc * P:(dc + 1) * P],
                                    ident_r[:])
            nc.vector.tensor_copy(
                aT[t][:, :, rc * P:(rc + 1) * P],
                ptb[:].bitcast(FMM).rearrange("p (c x) -> p c x", c=DC))
        a_r[t] = None
        gT[t] = nt_pool.tile([P, DC, NTW], FMM, tag="gT", name="gT")
        for d2c in range(DC):
            pg = ps_sc.tile([P, NTW], F32, tag="sc")
            for d1c in range(DC):
                nc.tensor.matmul(pg[:], m_sb[:, d1c, d2c * P:(d2c + 1) * P],
                                 aT[t][:, d1c, :],
                                 start=(d1c == 0), stop=(d1c == DC - 1))
            if d2c % 2 == 0:
                nc.vector.tensor_copy(gT[t][:, d2c, :], pg[:])
            else:
                nc.scalar.copy(gT[t][:, d2c, :], pg[:])
        aT[t] = None

    def vid_round(b, mc):
        """Round-copy one video chunk into video_r (DVE/GPSIMD split)."""
        nc.vector.tensor_copy(video_r[b][:, mc, :], vtmp[b][mc][:])
        if has_ba:
            wsc = small.tile([P, D], F32, tag="wsc")
            nc.vector.tensor_mul(wsc[:], vtmp[b][mc][:], q_bc[:])
            nc.vector.reduce_sum(w_col[b][:, mc, :], wsc[:],
                                 axis=mybir.AxisListType.X)
        vtmp[b][mc] = None

    def vid_tp(b, mc):
        """PE-transpose one rounded video chunk into videoT (ganged)."""
        ptb = ps_tp.tile([P, NTW], F32, tag="tp")
        for dc in range(DC):
            nc.tensor.transpose(ptb[:, dc * P:(dc + 1) * P].bitcast(FMM),
                                video_r[b][:, mc, dc * P:(dc + 1) * P],
                                ident_r[:])
        nc.vector.tensor_copy(
            videoT_t[:, :, mc * P:(mc + 1) * P],
            ptb[:].bitcast(FMM).rearrange("p (c x) -> p c x", c=DC))

    def body(t):
        """scores -> exp -> rowsum-acc -> attT -> rowsum reduce, att evict."""
        b, nt = divmod(t, NT)
        first = nt == 0
        if first:
            for mc0 in range(3):
                vid_round(b, mc0)
                vid_tp(b, mc0)
        exp_t = [None] * MC
        acc = small.tile([P, NTW], FMM, tag="acc")
        pa = [ps_att.tile([P, NTW], F32, tag="att", name="att")
              for _ in range(DC)]

        def attT(mc):
            for dc in range(DC):
                nc.tensor.matmul(pa[dc][:],
                                 video_r[b][:, mc, dc * P:(dc + 1) * P],
                                 exp_t[mc][:],
                                 start=(mc == 0), stop=(mc == MC - 1))

        for mc in range(MC):
            if first and mc + 3 < MC:
                vid_round(b, mc + 3)
                vid_tp(b, mc + 3)
            psc = ps_sc.tile([P, NTW], F32, tag="sc")
            for d2c in range(DC):
                nc.tensor.matmul(psc[:], videoT_t[:, d2c, mc * P:(mc + 1) * P],
                                 gT[t][:, d2c, :],
                                 start=(d2c == 0), stop=(d2c == DC - 1))
            exp_t[mc] = exp_pool.tile([P, NTW], FMM, tag="exp_t", name="exp_t")
            nc.scalar.activation(exp_t[mc][:], psc[:],
                                 mybir.ActivationFunctionType.Exp,
                                 bias=(w_col[b][:, mc, :] if has_ba else 0.0),
                                 scale=SCALE)
            if mc == 0:
                nc.vector.tensor_copy(acc[:], exp_t[mc][:].bitcast(F32))
            else:
                nc.vector.tensor_add(acc[:], acc[:].bitcast(F32),
                                     exp_t[mc][:].bitcast(F32))
            if mc > 1:
                attT(mc - 2)
        attT(MC - 2)
        attT(MC - 1)
        gT[t] = None
        if t + 1 < TOT:
            aud_round(t + 1)
        # rowsum reduce + eviction emitted now so the ACT copy overlaps
        # the next tile's prep matmuls (columnize happens in finish)
        prs = ps_sc.tile([P, NTW], F32, tag="sc")
        nc.tensor.matmul(prs[0:1, :], ones_col[:], acc[:],
                         start=True, stop=True)
        rs_row = small.tile([1, NTW], F32, tag="rs_row", bufs=1, name="rs_row")
        nc.scalar.copy(rs_row[:], prs[0:1, :])
        # evict att banks (split DVE/ACT)
        att_sb = nt_pool.tile([P, DC, NTW], FMM, tag="att_sb", bufs=1,
                              name="att_sb")
        for dc in range(DC):
            if dc % 2 == 0:
                nc.vector.tensor_copy(att_sb[:, dc, :], pa[dc][:])
            else:
                nc.scalar.copy(att_sb[:, dc, :], pa[dc][:])
        return rs_row, att_sb

    def finish(t, rs_row, att_sb):
        """reciprocal + out projection (+ output DMA on scalar queue)."""
        b, nt = divmod(t, NT)
        n0 = b * SEQ + nt * NTW
        rs_col = small.tile([P, NSC], F32, tag="rs_col")
        for ns in range(NSC):
            prc = ps_tp.tile([P, NTW], F32, tag="tp")
            nc.tensor.matmul(prc[:, 0:1], rs_row[:, ns * P:(ns + 1) * P],
                             ones_f32[0:1, 0:1], start=True, stop=True)
            nc.vector.tensor_copy(rs_col[:, ns:ns + 1], prc[:, 0:1])
        recip_col = small.tile([P, NSC], F32, tag="recip_col")
        nc.vector.reciprocal(recip_col[:], rs_col[:])
        # out projection; DVE fuses *recip and +b_o
        for ns in range(NSC):
            po = ps_sc.tile([P, NTW], F32, tag="sc")
            for ec in range(DC):
                nc.tensor.matmul(po[:], att_sb[:, ec, ns * P:(ns + 1) * P],
                                 woT[:, ec, :], start=(ec == 0),
                                 stop=(ec == DC - 1))
            o_sb = outp.tile([P, D], F32, tag="o_sb")
            nc.vector.scalar_tensor_tensor(o_sb[:], po[:],
                                           recip_col[:, ns:ns + 1], bo_bc[:],
                                           op0=mybir.AluOpType.mult,
                                           op1=mybir.AluOpType.add)
            if ns % 2 == 0:
                nc.scalar.dma_start(out[n0 + ns * P:n0 + (ns + 1) * P, :],
                                    o_sb[:])
            else:
                nc.sync.dma_start(out[n0 + ns * P:n0 + (ns + 1) * P, :],
                                  o_sb[:])

    # ---- software-pipelined flat loop ----
    vid_alloc(0)
    aud_round(0)
    prep(0)
    for t in range(TOT):
        if t + 1 < TOT:
            aud_dma(t + 1)
        rs_row, att_sb = body(t)
        if t == 0:
            wo_build()
        if t + 1 < TOT:
            if (t + 1) % NT == 0:
                b_next = (t + 1) // NT
                vid_dma(b_next)
                vid_alloc(b_next)
            prep(t + 1)
        finish(t, rs_row, att_sb)


_NC_CACHE = {}


def _build(has_ba=False):
    if has_ba in _NC_CACHE:
        return _NC_CACHE[has_ba]
    nc = bacc.Bacc("TRN2", target_bir_lowering=False, debug=False,
                   num_devices=NCORES)
    nc.t_audio = nc.dram_tensor("audio", [BL * SEQ, D], F32, kind="ExternalInput")
    nc.t_video = nc.dram_tensor("video", [BL * SEQ, D], F32, kind="ExternalInput")
    nc.t_wa = nc.dram_tensor("w_a", [D, D], F32, kind="ExternalInput")
    nc.t_wv = nc.dram_tensor("w_v", [D, D], F32, kind="ExternalInput")
    nc.t_wo = nc.dram_tensor("w_o", [D, D], F32, kind="ExternalInput")
    nc.t_ba = nc.dram_tensor("b_a", [D], F32, kind="ExternalInput")
    nc.t_bo = nc.dram_tensor("b_o", [D], F32, kind="ExternalInput")
    nc.t_out = nc.dram_tensor("out", [BL * SEQ, D], F32, kind="ExternalOutput")
    with tile.TileContext(nc) as tc:
        with ExitStack() as ctx:
            _body(tc, ctx, has_ba=has_ba)
    nc.compile()
    _NC_CACHE[has_ba] = nc
    return nc


def kernel(audio, video, W_a, b_a, W_v, b_v, W_o, b_o, _trace=False):
    nc = _build(has_ba=bool(np.any(np.asarray(b_a))))
    audio = np.ascontiguousarray(audio, dtype=np.float32)
    video = np.ascontiguousarray(video, dtype=np.float32)
    shared = {
        "w_a": np.ascontiguousarray(W_a, dtype=np.float32),
        "w_v": np.ascontiguousarray(W_v, dtype=np.float32),
        "w_o": np.ascontiguousarray(W_o, dtype=np.float32),
        "b_a": np.ascontiguousarray(b_a, dtype=np.float32),
        "b_o": np.ascontiguousarray(b_o, dtype=np.float32),
    }
    in_maps = []
    for c in range(NCORES):
        sl = slice(c * BL, (c + 1) * BL)
        in_maps.append({
            "audio": audio[sl].reshape(BL * SEQ, D),
            "video": video[sl].reshape(BL * SEQ, D),
            **shared,
        })
    res = run_bass_kernel_spmd(nc, in_maps, core_ids=list(range(NCORES)),
                               trace=_trace)
    out = np.concatenate(
        [res.results[c]["out"].reshape(BL, SEQ, D) for c in range(NCORES)],
        axis=0)
    if _trace:
        kernel.last_exec_time_ns = res.exec_time_ns
        kernel.last_results = res
    return out


# revision 25
# speedup vs baseline: 1.2037x; 1.2037x over previous
"""CrossModalAttention kernel for 8 Trainium2 NeuronCores.

Data-parallel over batch: B=16 -> 2 batches per core.

Math (per batch, with A=audio [N,d], V=video [M,d]):
  scores*sqrt(d) = (A Wa^T + 1 b_a^T)(V Wv^T + 1 b_v^T)^T
                 = A M V^T + (row-constant terms) + 1_n w^T
  with M = Wa^T Wv, w = V (Wv^T b_a).  Row-constant terms drop inside
  softmax, and max-subtraction is skipped: scores are ~N(0,1), far from
  fp32 exp overflow.
  attn     = exp_s / rowsum, with exp_s kept transposed [m, n]
  att_T[d,n] = sum_m video[m,d] exp_s[m,n]
  out[n,f] = att_T^T @ Wo^T * (1/rowsum[n]) + b_o

All matmuls run as fp32r (TF32-like, 1 cyc/row) with fp32 PSUM
accumulation; fp32r operands are produced by explicit rounding copies
(BIR verifier requirement).  K is always on partitions:

  video_r[m,d] <- GPSIMD round-copy of DMA'd video chunk
  videoT[d,m]  <- PE-transpose of video_r (fp32r, 1.5 cyc/row)
  aT[d1,n]  <- PE-transpose of audio slice (fp32, 2 cyc/row, cast on evict)
  gT[d2,n]  =  M[d1,d2](st)       x aT(mv)
  sT[m,n]   =  videoT[d2,m](st)   x gT(mv);   exp on ACT -> fp32r
  rowsum    :  DVE-accumulated exp chunks, one [1,n] PE reduce at end
  attT[d,n] =  video_r[m,d](st)   x exp[m](mv), 4 PSUM banks held per nt
  out[n,f]  =  attT[e,n](st)      x WoT[e,f](mv); DVE fuses *recip + b_o

Scheduling: flat software-pipelined loop over the 8 (batch, n-tile)
pairs.  Video prep (chunk DMA + round + PE transpose) is fused into
each batch's first n-tile mc-loop so the PE never sits idle long
enough for the HAM clock gate to re-throttle.  Audio/gT prep for tile
t+1 is emitted between tile t's attention and its output projection.
DMA issue queues: video on Sync, audio on Scalar, output on GpSimd.
"""

import os
from contextlib import ExitStack

import numpy as np

import concourse.bass as bass
import concourse.mybir as mybir
import concourse.tile as tile
from concourse import bacc
from concourse.bass_utils import run_bass_kernel_spmd
from concourse.masks import make_identity

B, SEQ, D = 16, 2048, 512
NCORES = 8
BL = B // NCORES          # batches per core
P = 128
DC = D // P               # 4 chunks of the model dim
MC = SEQ // P             # 16 m-chunks per batch
NTW = 512                 # n-tile width
NT = SEQ // NTW           # 4 n-tiles per batch
NSC = NTW // P            # 4 n-subchunks per n-tile
TOT = BL * NT             # flat (batch, nt) tiles per core
SCALE = 1.0 / float(np.sqrt(D))

F32 = mybir.dt.float32
FMM = mybir.dt.float32r   # matmul operand dtype


def _body(tc, ctx, has_ba=False):
    nc = tc.nc
    audio = nc.t_audio.ap()
    video = nc.t_video.ap()
    out = nc.t_out.ap()

    const = ctx.enter_context(tc.tile_pool(name="const", bufs=1))
    # PSUM: 4 att banks held per nt + 2 rotating big banks + 2 transpose
    ps_att = ctx.enter_context(tc.tile_pool(name="ps_att", bufs=4, space="PSUM"))
    ps_sc = ctx.enter_context(tc.tile_pool(name="ps_sc", bufs=2, space="PSUM"))
    ps_tp = ctx.enter_context(tc.tile_pool(name="ps_tp", bufs=2, space="PSUM"))

    # ---- constants ----
    ident = const.tile([P, P], F32, tag="ident")
    make_identity(nc, ident[:])
    ident_r = const.tile([P, P], FMM, tag="ident_r")
    nc.vector.tensor_copy(ident_r[:], ident[:])
    ones_f32 = const.tile([P, P], F32, tag="ones_f32")
    nc.gpsimd.memset(ones_f32[:], 1.0)
    ones_col = const.tile([P, 1], FMM, tag="ones_col")
    nc.vector.tensor_copy(ones_col[:], ones_f32[:, 0:1])
    ones_row = const.tile([1, P], FMM, tag="ones_row")
    nc.vector.tensor_copy(ones_row[:], ones_f32[0:1, :])

    m_sb = const.tile([P, DC, D], FMM, tag="m_sb")
    woT = const.tile([P, DC, D], FMM, tag="woT")
    wo_sb = const.tile([P, DC, D], F32, tag="wo_sb")
    bo_sb = const.tile([1, D], F32, tag="bo_sb")
    bo_bc = const.tile([P, D], F32, tag="bo_bc")
    q_bc = const.tile([P, D], F32, tag="q_bc") if has_ba else None

    # early pools so data DMAs can be issued during setup
    vtmp_pool = ctx.enter_context(tc.tile_pool(name="vtmp", bufs=6))
    nt_pool = ctx.enter_context(tc.tile_pool(name="nt", bufs=2))

    vtmp = [[None] * MC for _ in range(BL)]
    a_sb = [None] * TOT
    a_r = [None] * TOT
    aT = [None] * TOT
    gT = [None] * TOT

    def vid_dma(b):
        """Issue video chunk DMAs for batch b (sync queue)."""
        b0 = b * SEQ
        for mc in range(MC):
            vt = vtmp_pool.tile([P, D], F32, tag="vtmp", name="vtmp")
            nc.sync.dma_start(vt[:], video[b0 + mc * P:b0 + (mc + 1) * P, :])
            vtmp[b][mc] = vt

    def aud_dma(t):
        """Issue audio DMAs for flat tile t (gpsimd queue)."""
        a_sb[t] = nt_pool.tile([P, NSC, D], F32, tag="a_sb", bufs=1,
                               name="a_sb")
        a_r[t] = nt_pool.tile([P, NSC, D], FMM, tag="a_r", bufs=1, name="a_r")
        n0 = (t // NT) * SEQ + (t % NT) * NTW
        for rc in range(NSC):
            nc.gpsimd.dma_start(a_sb[t][:, rc, :],
                                audio[n0 + rc * P:n0 + (rc + 1) * P, :])

    def aud_round(t):
        """DVE round-casts audio to fp32r (emitted once the DMA is old)."""
        for rc in range(NSC):
            nc.vector.tensor_copy(a_r[t][:, rc, :], a_sb[t][:, rc, :])
        a_sb[t] = None

    # ---- setup: weights, M = Wa^T Wv, WoT, biases ----
    # wa on sync / wv on scalar so both rings run in parallel; the data
    # DMAs (video chunks on sync, audio on scalar) queue right behind.
    with tc.tile_pool(name="setup", bufs=1) as setup:
        wa_sb = setup.tile([P, DC, D], F32, tag="wa_sb")
        wv_sb = setup.tile([P, DC, D], F32, tag="wv_sb")
        # per-chunk weight DMAs so the round copies / M matmuls can start
        # on chunk 0 while later chunks are still in flight
        for ec in range(DC):
            nc.sync.dma_start(wa_sb[:, ec, :],
                              nc.t_wa.ap()[ec * P:(ec + 1) * P, :])
            nc.scalar.dma_start(wv_sb[:, ec, :],
                                nc.t_wv.ap()[ec * P:(ec + 1) * P, :])
        vid_dma(0)
        aud_dma(0)
        nc.gpsimd.dma_start(bo_sb[:], nc.t_bo.ap().rearrange("(o d) -> o d", o=1))
        for fc in range(DC):
            nc.gpsimd.dma_start(wo_sb[:, fc, :],
                                nc.t_wo.ap()[fc * P:(fc + 1) * P, :])

        # per-ec round copies pipelined with the M accumulation (ps_att)
        wa_r = setup.tile([P, DC, D], FMM, tag="wa_r")
        wv_r = setup.tile([P, DC, D], FMM, tag="wv_r")
        pm = [ps_att.tile([P, NTW], F32, tag="att", name="pm")
              for _ in range(DC)]
        for ec in range(DC):
            nc.scalar.copy(wa_r[:, ec, :], wa_sb[:, ec, :])
            nc.vector.tensor_copy(wv_r[:, ec, :], wv_sb[:, ec, :])
            for d1c in range(DC):
                nc.tensor.matmul(pm[d1c][:], wa_r[:, ec, d1c * P:(d1c + 1) * P],
                                 wv_r[:, ec, :], start=(ec == 0),
                                 stop=(ec == DC - 1))
        for d1c in range(DC):
            nc.scalar.copy(m_sb[:, d1c, :], pm[d1c][:])

        if has_ba:
            ba_sb = setup.tile([P, DC], F32, tag="ba_sb")
            nc.sync.dma_start(ba_sb[:], nc.t_ba.ap().rearrange("(c p) -> p c", p=P))
            ba_r = setup.tile([P, DC], FMM, tag="ba_r")
            nc.vector.tensor_copy(ba_r[:], ba_sb[:])
            # q = Wv^T b_a [1, d] -> broadcast to q_bc [128, d], pre-scaled
            pqf = ps_sc.tile([P, NTW], F32, tag="sc")
            for ec in range(DC):
                nc.tensor.matmul(pqf[0:1, :], ba_r[:, ec:ec + 1],
                                 wv_r[:, ec, :],
                                 start=(ec == 0), stop=(ec == DC - 1))
            q_row = setup.tile([1, D], FMM, tag="q_row")
            nc.scalar.mul(q_row[:], pqf[0:1, :], SCALE)
            pqb = ps_sc.tile([P, NTW], F32, tag="sc")
            nc.tensor.matmul(pqb[:], ones_row[:], q_row[:], start=True, stop=True)
            nc.vector.tensor_copy(q_bc[:], pqb[:])

    # ---- main pools ----
    vid = ctx.enter_context(tc.tile_pool(name="vid", bufs=1))
    exp_pool = ctx.enter_context(tc.tile_pool(name="expp", bufs=4))
    outp = ctx.enter_context(tc.tile_pool(name="outp", bufs=2))
    small = ctx.enter_context(tc.tile_pool(name="small", bufs=2))

    # per-batch video buffers (round-copied fp32r, double-buffered)
    video_r = [None] * BL
    videoT_t = vid.tile([P, DC, SEQ], FMM, tag="videoT")
    w_col = [None] * BL

    bo_row_t = small.tile([1, D], FMM, tag="bo_row", bufs=1, name="bo_row")

    def wo_build():
        """woT transposes + b_o broadcast; emitted after body(0) so the
        PE is already warm and the wo DMA is long done."""
        for fc in range(DC):
            ptb = ps_tp.tile([P, NTW], F32, tag="tp")
            for ec in range(DC):
                nc.tensor.transpose(ptb[:, ec * P:(ec + 1) * P],
                                    wo_sb[:, fc, ec * P:(ec + 1) * P],
                                    ident[:])
            nc.vector.tensor_copy(
                woT[:, :, fc * P:(fc + 1) * P],
                ptb[:].rearrange("p (c x) -> p c x", c=DC))
        nc.vector.tensor_copy(bo_row_t[:], bo_sb[:])
        pbb = ps_sc.tile([P, NTW], F32, tag="sc")
        nc.tensor.matmul(pbb[:], ones_row[:], bo_row_t[:], start=True, stop=True)
        nc.vector.tensor_copy(bo_bc[:], pbb[:])

    def vid_alloc(b):
        video_r[b] = vid.tile([P, MC, D], FMM, tag="video_r", bufs=2,
                              name="video_r")
        if has_ba:
            w_col[b] = vid.tile([P, MC, 1], F32, tag="w_col", bufs=2,
                                name="w_col")

    def prep(t):
        """Audio transposes + gT for flat tile t (PE work).  The 4 dc
        transposes of one row-chunk go into a single full PSUM bank and
        leave with one strided eviction: back-to-back PE transposes and
        4x fewer DVE round-trips on the ps_tp ring."""
        aT[t] = nt_pool.tile([P, DC, NTW], FMM, tag="aT", bufs=1, name="aT")
        for rc in range(NSC):
            ptb = ps_tp.tile([P, NTW], F32, tag="tp")
            for dc in range(DC):
                nc.tensor.transpose(ptb[:, dc * P:(dc + 1) * P].bitcast(FMM),
                                    a_r[t][:, rc, dc * P:(dc + 1) * P],
                                    ident_r[:])
            nc.vector.tensor_copy(
                aT[t][:, :, rc * P:(rc + 1) * P],
                ptb[:].bitcast(FMM).rearrange("p (c x) -> p c x", c=DC))
        a_r[t] = None
        gT[t] = nt_pool.tile([P, DC, NTW], FMM, tag="gT", name="gT")
        for d2c in range(DC):
            pg = ps_sc.tile([P, NTW], F32, tag="sc")
            for d1c in range(DC):
                nc.tensor.matmul(pg[:], m_sb[:, d1c, d2c * P:(d2c + 1) * P],
                                 aT[t][:, d1c, :],
                                 start=(d1c == 0), stop=(d1c == DC - 1))
            if d2c % 2 == 0:
                nc.vector.tensor_copy(gT[t][:, d2c, :], pg[:])
            else:
                nc.scalar.copy(gT[t][:, d2c, :], pg[:])
        aT[t] = None

    def vid_round(b, mc):
        """Round-copy one video chunk into video_r (DVE/GPSIMD split)."""
        nc.vector.tensor_copy(video_r[b][:, mc, :], vtmp[b][mc][:])
        if has_ba:
            wsc = small.tile([P, D], F32, tag="wsc")
            nc.vector.tensor_mul(wsc[:], vtmp[b][mc][:], q_bc[:])
            nc.vector.reduce_sum(w_col[b][:, mc, :], wsc[:],
                                 axis=mybir.AxisListType.X)
        vtmp[b][mc] = None

    def vid_tp(b, mc):
        """PE-transpose one rounded video chunk into videoT (ganged)."""
        ptb = ps_tp.tile([P, NTW], F32, tag="tp")
        for dc in range(DC):
            nc.tensor.transpose(ptb[:, dc * P:(dc + 1) * P].bitcast(FMM),
                                video_r[b][:, mc, dc * P:(dc + 1) * P],
                                ident_r[:])
        nc.vector.tensor_copy(
            videoT_t[:, :, mc * P:(mc + 1) * P],
            ptb[:].bitcast(FMM).rearrange("p (c x) -> p c x", c=DC))

    def body(t):
        """scores -> exp -> rowsum-acc -> attT -> rowsum reduce, att evict."""
        b, nt = divmod(t, NT)
        first = nt == 0
        if first:
            for mc0 in range(3):
                vid_round(b, mc0)
                vid_tp(b, mc0)
        exp_t = [None] * MC
        acc = small.tile([P, NTW], FMM, tag="acc")
        pa = [ps_att.tile([P, NTW], F32, tag="att", name="att")
              for _ in range(DC)]

        def attT(mc):
            for dc in range(DC):
                nc.tensor.matmul(pa[dc][:],
                                 video_r[b][:, mc, dc * P:(dc + 1) * P],
                                 exp_t[mc][:],
                                 start=(mc == 0), stop=(mc == MC - 1))

        for mc in range(MC):
            if first and mc + 3 < MC:
                vid_round(b, mc + 3)
                vid_tp(b, mc + 3)
            psc = ps_sc.tile([P, NTW], F32, tag="sc")
            for d2c in range(DC):
                nc.tensor.matmul(psc[:], videoT_t[:, d2c, mc * P:(mc + 1) * P],
                                 gT[t][:, d2c, :],
                                 start=(d2c == 0), stop=(d2c == DC - 1))
            exp_t[mc] = exp_pool.tile([P, NTW], FMM, tag="exp_t", name="exp_t")
            nc.scalar.activation(exp_t[mc][:], psc[:],
                                 mybir.ActivationFunctionType.Exp,
                                 bias=(w_col[b][:, mc, :] if has_ba else 0.0),
                                 scale=SCALE)
            if mc == 0:
                nc.vector.tensor_copy(acc[:], exp_t[mc][:].bitcast(F32))
            else:
                nc.vector.tensor_add(acc[:], acc[:].bitcast(F32),
                                     exp_t[mc][:].bitcast(F32))
            if mc > 1:
                attT(mc - 2)
        attT(MC - 2)
        attT(MC - 1)
        gT[t] = None
        if t + 1 < TOT:
            aud_round(t + 1)
        # rowsum reduce + eviction emitted now so the ACT copy overlaps
        # the next tile's prep matmuls (columnize happens in finish)
        prs = ps_sc.tile([P, NTW], F32, tag="sc")
        nc.tensor.matmul(prs[0:1, :], ones_col[:], acc[:],
                         start=True, stop=True)
        rs_row = small.tile([1, NTW], F32, tag="rs_row", bufs=1, name="rs_row")
        nc.scalar.copy(rs_row[:], prs[0:1, :])
        # evict att banks (split DVE/ACT)
        att_sb = nt_pool.tile([P, DC, NTW], FMM, tag="att_sb", bufs=1,
                              name="att_sb")
        for dc in range(DC):
            if dc % 2 == 0:
                nc.vector.tensor_copy(att_sb[:, dc, :], pa[dc][:])
            else:
                nc.scalar.copy(att_sb[:, dc, :], pa[dc][:])
        return rs_row, att_sb

    def finish(t, rs_row, att_sb):
        """reciprocal + out projection (+ output DMA on scalar queue)."""
        b, nt = divmod(t, NT)
        n0 = b * SEQ + nt * NTW
        rs_col = small.tile([P, NSC], F32, tag="rs_col")
        for ns in range(NSC):
            prc = ps_tp.tile([P, NTW], F32, tag="tp")
            nc.tensor.matmul(prc[:, 0:1], rs_row[:, ns * P:(ns + 1) * P],
                             ones_f32[0:1, 0:1], start=True, stop=True)
            nc.vector.tensor_copy(rs_col[:, ns:ns + 1], prc[:, 0:1])
        recip_col = small.tile([P, NSC], F32, tag="recip_col")
        nc.vector.reciprocal(recip_col[:], rs_col[:])
        # out projection; DVE fuses *recip and +b_o
        for ns in range(NSC):
            po = ps_sc.tile([P, NTW], F32, tag="sc")
            for ec in range(DC):
                nc.tensor.matmul(po[:], att_sb[:, ec, ns * P:(ns + 1) * P],
                                 woT[:, ec, :], start=(ec == 0),
                                 stop=(ec == DC - 1))
            o_sb = outp.tile([P, D], F32, tag="o_sb")
            nc.vector.scalar_tensor_tensor(o_sb[:], po[:],
                                           recip_col[:, ns:ns + 1], bo_bc[:],
                                           op0=mybir.AluOpType.mult,
                                           op1=mybir.AluOpType.add)
            if ns % 2 == 0:
                nc.scalar.dma_start(out[n0 + ns * P:n0 + (ns + 1) * P, :],
                                    o_sb[:])
            else:
                nc.sync.dma_start(out[n0 + ns * P:n0 + (ns + 1) * P, :],
                                  o_sb[:])

    # ---- software-pipelined flat loop ----
    vid_alloc(0)
    aud_round(0)
    prep(0)
    for t in range(TOT):
        if t + 1 < TOT:
            aud_dma(t + 1)
        rs_row, att_sb = body(t)
        if t == 0:
            wo_build()
        if t + 1 < TOT:
            if (t + 1) % NT == 0:
                b_next = (t + 1) // NT
                vid_dma(b_next)
                vid_alloc(b_next)
            prep(t + 1)
        finish(t, rs_row, att_sb)


_NC_CACHE = {}


def _build(has_ba=False):
    if has_ba in _NC_CACHE:
        return _NC_CACHE[has_ba]
    nc = bacc.Bacc("TRN2", target_bir_lowering=False, debug=False,
                   num_devices=NCORES)
    nc.t_audio = nc.dram_tensor("audio", [BL * SEQ, D], F32, kind="ExternalInput")
    nc.t_video = nc.dram_tensor("video", [BL * SEQ, D], F32, kind="ExternalInput")
    nc.t_wa = nc.dram_tensor("w_a", [D, D], F32, kind="ExternalInput")
    nc.t_wv = nc.dram_tensor("w_v", [D, D], F32, kind="ExternalInput")
    nc.t_wo = nc.dram_tensor("w_o", [D, D], F32, kind="ExternalInput")
    nc.t_ba = nc.dram_tensor("b_a", [D], F32, kind="ExternalInput")
    nc.t_bo = nc.dram_tensor("b_o", [D], F32, kind="ExternalInput")
    nc.t_out = nc.dram_tensor("out", [BL * SEQ, D], F32, kind="ExternalOutput")
    with tile.TileContext(nc) as tc:
        with ExitStack() as ctx:
            _body(tc, ctx, has_ba=has_ba)
    nc.compile()
    _NC_CACHE[has_ba] = nc
    return nc


def kernel(audio, video, W_a, b_a, W_v, b_v, W_o, b_o, _trace=False):
    nc = _build(has_ba=bool(np.any(np.asarray(b_a))))
    audio = np.ascontiguousarray(audio, dtype=np.float32)
    video = np.ascontiguousarray(video, dtype=np.float32)
    shared = {
        "w_a": np.ascontiguousarray(W_a, dtype=np.float32),
        "w_v": np.ascontiguousarray(W_v, dtype=np.float32),
        "w_o": np.ascontiguousarray(W_o, dtype=np.float32),
        "b_a": np.ascontiguousarray(b_a, dtype=np.float32),
        "b_o": np.ascontiguousarray(b_o, dtype=np.float32),
    }
    in_maps = []
    for c in range(NCORES):
        sl = slice(c * BL, (c + 1) * BL)
        in_maps.append({
            "audio": audio[sl].reshape(BL * SEQ, D),
            "video": video[sl].reshape(BL * SEQ, D),
            **shared,
        })
    res = run_bass_kernel_spmd(nc, in_maps, core_ids=list(range(NCORES)),
                               trace=_trace)
    out = np.concatenate(
        [res.results[c]["out"].reshape(BL, SEQ, D) for c in range(NCORES)],
        axis=0)
    if _trace:
        kernel.last_exec_time_ns = res.exec_time_ns
        kernel.last_results = res
    return out
